# revision 6
# baseline (speedup 1.0000x reference)
"""Batched 2048-point DFT on 8 TRN2 NeuronCores — four-step (Cooley-Tukey) version.

n = 2048 = 128 * 16, m = 16*m1 + m2, k = k1 + 128*k2:
  X[b, k1 + 128*k2] = sum_m2 W16[m2,k2] * ( sum_m1 A_m2[m1,k1] * x[b, 16*m1+m2] )
with A_m2[m1,k1] = exp(-2i*pi*(16*m1+m2)*k1/2048)  (128-DFT with the inter-stage
twiddle folded in — no separate twiddle pass).

Per core (batch shard of 512):
  stage 1: 16 matmuls pairs, stationary A_m2 (f32r), moving xR[m1,(m2,b)]
           -> Z[k1, (b, m2)] in SBUF (m2 innermost)
  stage 2: PE transposes of 64x2 [128,128] chunks -> T[(b_lo,m2), k1]
  stage 3: block-diag stationary S[(bl,m2),(bl,k2)] = W16[m2,k2] matmuls
           -> X[b, k] directly (4D-AP DMA scatter, no host transpose of output)
All heavy host prep (reshape/negation/twiddle build) is free host-side work.
"""

import sys

for _p in ("/opt/trn_rl_repo", "/root/.axon_site/_ro/trn_rl_repo"):
    if _p not in sys.path:
        sys.path.insert(0, _p)

import numpy as np

import concourse.bass as bass
import concourse.mybir as mybir
import concourse.tile as tile
from concourse import bacc
from concourse.bass_utils import run_bass_kernel_spmd
from concourse.masks import make_identity

BATCH = 4096
NFFT = 2048
NCORES = 8
BPC = BATCH // NCORES  # 512
N1 = 128
N2 = 16
NCHUNK = BPC * N2 // 128  # 64 transpose chunks per part
NGRP = NCHUNK // 4  # 16 stage-3 groups

F32 = mybir.dt.float32
F32R = mybir.dt.float32r

_CACHE = {}


def _build_nc():
    nc = bacc.Bacc("TRN2", target_bir_lowering=False, debug=False)

    # xR layout: [m2 16, m1 128, b 512] flat [2048, 512] (contiguous per-q DMA)
    xre_d = nc.dram_tensor("xre", [N2 * N1, BPC], F32, kind="ExternalInput").ap()
    xim_d = nc.dram_tensor("xim", [N2 * N1, BPC], F32, kind="ExternalInput").ap()
    # A stationaries: [m2, m1 128, 3(re, im, imneg), k1 128] flat [2048, 384]
    a_d = nc.dram_tensor("amat", [N2 * 128, 3 * 128], F32, kind="ExternalInput").ap()
    # S block-diag: [3, 128, 128]
    s_d = nc.dram_tensor("smat", [3 * 128, 128], F32, kind="ExternalInput").ap()
    ore_d = nc.dram_tensor("ore", [BPC, NFFT], F32, kind="ExternalOutput").ap()
    oim_d = nc.dram_tensor("oim", [BPC, NFFT], F32, kind="ExternalOutput").ap()

    a_v = a_d.bitcast(F32R).rearrange("(q p) (v k) -> q p v k", q=N2, v=3)
    s_v = s_d.bitcast(F32R).rearrange("(v p) k -> v p k", v=3)

    with tile.TileContext(nc) as tc:
        with (
            tc.tile_pool(name="const", bufs=1) as cpool,
            tc.tile_pool(name="x", bufs=1) as xpool,
            tc.tile_pool(name="z", bufs=1) as zpool,
            tc.tile_pool(name="t", bufs=3) as tpool,
            tc.tile_pool(name="o", bufs=3) as opool,
            tc.tile_pool(name="ps1", bufs=3, space="PSUM") as ps1pool,
            tc.tile_pool(name="pst", bufs=3, space="PSUM") as pstpool,
            tc.tile_pool(name="ps2", bufs=2, space="PSUM") as ps2pool,
        ):
            # constants + resident moving operand, interleaved for startup
            a_t = cpool.tile([128, N2, 3, 128], F32R, tag="amat")
            xre_t = xpool.tile([128, N2, BPC], F32R, tag="xre")
            xim_t = xpool.tile([128, N2, BPC], F32R, tag="xim")
            xre_v = xre_d.bitcast(F32R).rearrange("(q p) b -> q p b", q=N2)
            xim_v = xim_d.bitcast(F32R).rearrange("(q p) b -> q p b", q=N2)
            for q in range(N2):
                nc.sync.dma_start(a_t[:, q, :, :], a_v[q])
                nc.sync.dma_start(xre_t[:, q, :], xre_v[q])
                nc.sync.dma_start(xim_t[:, q, :], xim_v[q])
            s_t = cpool.tile([128, 3, 128], F32R, tag="smat")
            nc.sync.dma_start(s_t[:], s_v.transpose([1, 0, 2]))
            ident = cpool.tile([128, 128], F32, tag="ident")
            make_identity(nc, ident[:])

            # stage-1 output, col = b*16 + m2 (m2 innermost: transpose chunks
            # contiguous, stage-1 psum copies strided by 16)
            z_re = zpool.tile([128, BPC * N2], F32, tag="zre")
            z_im = zpool.tile([128, BPC * N2], F32, tag="zim")
            z_re3 = z_re[:].rearrange("p (b q) -> p b q", q=N2)
            z_im3 = z_im[:].rearrange("p (b q) -> p b q", q=N2)

            # ---- stage 1 ----
            for q in range(N2):
                ps_re = ps1pool.tile([128, BPC], F32, tag="ps1")
                ps_im = ps1pool.tile([128, BPC], F32, tag="ps1")
                are = a_t[:, q, 0, :]
                aim = a_t[:, q, 1, :]
                aimn = a_t[:, q, 2, :]
                nc.tensor.matmul(ps_re[:], are, xre_t[:, q, :], start=True, stop=False)
                nc.tensor.matmul(ps_im[:], are, xim_t[:, q, :], start=True, stop=False)
                nc.tensor.matmul(ps_im[:], aim, xre_t[:, q, :], start=False, stop=True)
                nc.tensor.matmul(ps_re[:], aimn, xim_t[:, q, :], start=False, stop=True)
                nc.vector.tensor_copy(z_re3[:, :, q], ps_re[:])
                nc.scalar.copy(z_im3[:, :, q], ps_im[:])

            # ---- stage 2 + 3, per group of 4 chunks ----
            sre = s_t[:, 0, :]
            sim = s_t[:, 1, :]
            simn = s_t[:, 2, :]
            for g in range(NGRP):
                t_re = tpool.tile([128, 512], F32R, tag="tre")
                t_im = tpool.tile([128, 512], F32R, tag="tim")
                pt_re = pstpool.tile([128, 512], F32, tag="pt")
                pt_im = pstpool.tile([128, 512], F32, tag="pt")
                for j in range(4):
                    c = g * 4 + j
                    csl = slice(c * 128, (c + 1) * 128)
                    jsl = slice(j * 128, (j + 1) * 128)
                    nc.tensor.transpose(pt_re[:, jsl], z_re[:, csl], ident[:])
                    nc.tensor.transpose(pt_im[:, jsl], z_im[:, csl], ident[:])
                nc.vector.tensor_copy(t_re[:], pt_re[:])
                nc.scalar.copy(t_im[:], pt_im[:])

                ps2_re = ps2pool.tile([128, 512], F32, tag="ps2")
                ps2_im = ps2pool.tile([128, 512], F32, tag="ps2")
                nc.tensor.matmul(ps2_re[:], sre, t_re[:], start=True, stop=False)
                nc.tensor.matmul(ps2_im[:], sre, t_im[:], start=True, stop=False)
                nc.tensor.matmul(ps2_im[:], sim, t_re[:], start=False, stop=True)
                nc.tensor.matmul(ps2_re[:], simn, t_im[:], start=False, stop=True)

                o_re = opool.tile([128, 512], F32, tag="ore")
                o_im = opool.tile([128, 512], F32, tag="oim")
                nc.vector.tensor_copy(o_re[:], ps2_re[:])
                nc.scalar.copy(o_im[:], ps2_im[:])

                # scatter: partition p=(bl,kt), col=(j,ko); b = g*32+j*8+bl, k = kt*128+ko
                dst_re = ore_d.rearrange(
                    "(g j bl) (kt ko) -> g (bl kt) j ko", g=NGRP, j=4, bl=8, kt=N2
                )[g]
                dst_im = oim_d.rearrange(
                    "(g j bl) (kt ko) -> g (bl kt) j ko", g=NGRP, j=4, bl=8, kt=N2
                )[g]
                nc.sync.dma_start(dst_re, o_re[:].rearrange("p (j ko) -> p j ko", j=4))
                nc.sync.dma_start(dst_im, o_im[:].rearrange("p (j ko) -> p j ko", j=4))

    nc.compile()
    return nc


def _consts():
    m1 = np.arange(N1, dtype=np.float64)
    k1 = np.arange(N1, dtype=np.float64)
    m2 = np.arange(N2, dtype=np.float64)
    k2 = np.arange(N2, dtype=np.float64)
    # A_m2[m1,k1] = exp(-2i pi (16 m1 + m2) k1 / 2048)
    a = np.empty((N2, 3, N1, N1), np.float32)
    for q in range(N2):
        ph = -2.0 * np.pi * np.outer(16.0 * m1 + q, k1) / NFFT
        a[q, 0] = np.cos(ph).astype(np.float32)
        a[q, 1] = np.sin(ph).astype(np.float32)
        a[q, 2] = -a[q, 1]
    # S[(bl,m2),(bl,k2)] = W16[m2,k2]
    ph16 = -2.0 * np.pi * np.outer(m2, k2) / N2
    w16re = np.cos(ph16).astype(np.float32)
    w16im = np.sin(ph16).astype(np.float32)
    s = np.zeros((3, 128, 128), np.float32)
    for bl in range(8):
        sl = slice(bl * 16, (bl + 1) * 16)
        s[0][sl, sl] = w16re
        s[1][sl, sl] = w16im
        s[2][sl, sl] = -w16im
    return (
        np.ascontiguousarray(a.transpose(0, 2, 1, 3).reshape(N2 * 128, 3 * 128)),
        np.ascontiguousarray(s.reshape(3 * 128, 128)),
    )


def run(signal_re, signal_im, trace=False, tmpdir=None):
    if "nc" not in _CACHE:
        _CACHE["nc"] = _build_nc()
        _CACHE["c"] = _consts()
    nc = _CACHE["nc"]
    amat, smat = _CACHE["c"]

    sre = np.asarray(signal_re, dtype=np.float32)
    sim = np.asarray(signal_im, dtype=np.float32)

    in_maps = []
    for c in range(NCORES):
        bsl = slice(c * BPC, (c + 1) * BPC)
        # xR[m1, m2, b]
        xre = np.ascontiguousarray(
            sre[bsl].reshape(BPC, N1, N2).transpose(2, 1, 0).reshape(N2 * N1, BPC)
        )
        xim = np.ascontiguousarray(
            sim[bsl].reshape(BPC, N1, N2).transpose(2, 1, 0).reshape(N2 * N1, BPC)
        )
        in_maps.append({"xre": xre, "xim": xim, "amat": amat, "smat": smat})

    # first execution of a fresh NEFF occasionally fails with a transient
    # INTERNAL runtime error; retry a couple of times before giving up
    last_exc = None
    for attempt in range(3):
        try:
            br = run_bass_kernel_spmd(
                nc, in_maps, list(range(NCORES)), trace=trace, tmpdir=tmpdir
            )
            break
        except Exception as e:
            last_exc = e
            import time

            time.sleep(2.0)
    else:
        raise last_exc

    out_re = np.empty((BATCH, NFFT), np.float32)
    out_im = np.empty((BATCH, NFFT), np.float32)
    for c in range(NCORES):
        bsl = slice(c * BPC, (c + 1) * BPC)
        out_re[bsl, :] = br.results[c]["ore"]
        out_im[bsl, :] = br.results[c]["oim"]
    return (out_re, out_im), br


def kernel(signal_re, signal_im):
    return run(signal_re, signal_im)[0]
